# revision 20
# baseline (speedup 1.0000x reference)
"""Trainium2 Bass kernel for cross "efficient attention".

Reference computation (per batch b, head h, with C=128, HEADS=8, hc=16, n=16384):
    k = x2[b].reshape(HEADS, hc, n); v = x1[b].reshape(HEADS, hc, n)
    key_sm   = softmax(k, axis=-1)          # over n
    query_sm = softmax(k, axis=1)           # over hc (head channels)
    context  = key_sm @ v^T                 # (hc, hc)
    out[b,h] = context^T @ query_sm         # (hc, n)

Sharding: data-parallel over batch B=8 across the 8 NeuronCores (no
collectives).  Inputs are ~N(0,1) so softmax needs no max-subtraction.

Layouts: x2 arrives natural [C, N] bf16; x1 arrives in "A-layout"
[128, N] bf16 (partition p holds n = j*128+p for every channel-block
j) so it is directly the n-on-partitions contraction operand.  All
DMAs are contiguous per partition.  HBM traffic 12 MiB/core.

Single streaming pipeline per slab of n:
    E    = exp(x2)               + rowsum accum        (scalar)
    eT   = PE-transpose(E)       -> SBUF               (PE + s/v copies)
    ctx += eT_j^T @ vT_j         (lagged 1 slab)       (PE)
    cs   = bd8 @ E               per-head colsums,
                                 broadcast for free    (PE -> PSUM)
    qsm  = E / cs                                      (vector divide)
Then bd = blockdiag(ctx * 1/rowsum) and attended tiles
    out[:, t] = bd^T @ qsm[:, t]                       (PE + s/v casts)
stream out.  The colsum and attended tiles share one PSUM pool ring so
the attended matmuls chain naturally behind the last colsum with no
pool-close barrier.
"""

import numpy as np
from contextlib import ExitStack

B, C, H, W = 8, 128, 128, 128
N = H * W                 # 16384
J = N // 128              # 128 channel-blocks
HEADS, HC = 8, 16
NCORES = 8

SLABS = [2048] * 7 + [1024, 512, 512]
assert sum(SLABS) == N
NSLAB = len(SLABS)
GRP = 1024                # transpose / colsum / divide chunk (PSUM bank-pair)
QS = 512                  # matmul moving-operand tile (one PSUM bank f32)
OT = 1024                 # attended tile width
NOT = N // OT             # 16

_cache: dict = {}


def _build():
    import concourse.bass as bass
    import concourse.tile as tile
    from concourse import bacc, mybir

    FP32 = mybir.dt.float32
    BF16 = mybir.dt.bfloat16
    AF = mybir.ActivationFunctionType

    nc = bacc.Bacc("TRN2", target_bir_lowering=False, debug=False)

    x2n = nc.dram_tensor("x2n", [C, N], BF16, kind="ExternalInput")
    x1p = nc.dram_tensor("x1p", [128, N], BF16, kind="ExternalInput")
    bd8_in = nc.dram_tensor("bd8", [C, C], BF16, kind="ExternalInput")
    ident_in = nc.dram_tensor("ident", [C, C], BF16, kind="ExternalInput")
    out = nc.dram_tensor("out", [C, N], BF16, kind="ExternalOutput")

    with tile.TileContext(nc) as tc:
        with ExitStack() as ctx:
            persist = ctx.enter_context(tc.tile_pool(name="persist", bufs=1))
            x2ld = ctx.enter_context(tc.tile_pool(name="x2ld", bufs=4))
            vld = ctx.enter_context(tc.tile_pool(name="vld", bufs=4))
            enp = ctx.enter_context(tc.tile_pool(name="enp", bufs=4))
            eTp = ctx.enter_context(tc.tile_pool(name="eTp", bufs=6))
            rcpp = ctx.enter_context(tc.tile_pool(name="rcpp", bufs=3))
            outp = ctx.enter_context(tc.tile_pool(name="outp", bufs=4))
            smalls = ctx.enter_context(tc.tile_pool(name="smalls", bufs=1))

            qsm_nat = persist.tile([C, N], BF16, tag="qsm_nat")
            bd8 = smalls.tile([C, C], BF16, tag="bd8")
            ident = smalls.tile([C, C], BF16, tag="ident")
            rs_acc = smalls.tile([C, NSLAB], FP32, tag="rs_acc")

            ps_tr = ctx.enter_context(
                tc.tile_pool(name="pstr", bufs=3, space="PSUM"))
            ps_ctx = ctx.enter_context(
                tc.tile_pool(name="psctx", bufs=1, space="PSUM"))
            # colsum tiles and attended tiles share this ring: the first
            # attended matmul naturally chains behind the last colsum's
            # consumer instead of a pool-close barrier.
            ps_mix = ctx.enter_context(
                tc.tile_pool(name="psmix", bufs=2, space="PSUM"))

            ctx_ps = ps_ctx.tile([C, 128], FP32, tag="ctx")

            mm_idx = 0
            pending = None       # (eT tiles, vT, n_chunks) of previous slab
            cpy = 0              # round-robin for PSUM->SBUF copy engine

            def emit_ctx(eTs, vT, nch):
                nonlocal mm_idx
                for j in range(nch):
                    nc.tensor.matmul(
                        ctx_ps[:],
                        eTs[j // 8][:, bass.ts(j % 8, 128)],
                        vT[:, bass.ts(j, 128)],
                        start=(mm_idx == 0),
                        stop=(mm_idx == J - 1),
                    )
                    mm_idx += 1

            off = 0
            for i, SW in enumerate(SLABS):
                Ji = SW // 128
                x2t = x2ld.tile([C, SW], BF16, tag="x2t")
                nc.sync.dma_start(out=x2t[:], in_=x2n[:, bass.ds(off, SW)])
                vT = vld.tile([128, SW], BF16, tag="vT")
                nc.sync.dma_start(out=vT[:], in_=x1p[:, bass.ds(off, SW)])
                if i == 0:
                    nc.scalar.dma_start(out=ident[:], in_=ident_in[:])
                    nc.scalar.dma_start(out=bd8[:], in_=bd8_in[:])

                en = enp.tile([C, SW], BF16, tag="en")
                nc.scalar.activation(
                    en[:], x2t[:], AF.Exp, accum_out=rs_acc[:, i:i + 1])

                # per-head colsums on the PE: bd8 is block-diagonal ones, so
                # the result arrives already broadcast to all 16 channels.
                eTs = []
                for g0 in range(0, SW, GRP):
                    gw = min(GRP, SW - g0)
                    cs = ps_mix.tile([C, GRP], FP32, tag="mix")
                    for q0 in range(0, gw, QS):
                        qw = min(QS, gw - q0)
                        nc.tensor.matmul(
                            cs[:, bass.ds(q0, qw)], bd8[:],
                            en[:, bass.ds(g0 + q0, qw)],
                        )
                    # transposes of the same chunk -> eT for the ctx matmul
                    te = ps_tr.tile([C, GRP], BF16, tag="te")
                    for j in range(gw // 128):
                        nc.tensor.transpose(
                            te[:, bass.ts(j, 128)],
                            en[:, bass.ds(g0 + j * 128, 128)],
                            ident[:],
                        )
                    eT = eTp.tile([C, GRP], BF16, tag="eT")
                    if cpy % 2 == 0:
                        nc.scalar.copy(eT[:, 0:gw], te[:, 0:gw])
                    else:
                        nc.vector.tensor_copy(eT[:, 0:gw], te[:, 0:gw])
                    cpy += 1
                    eTs.append(eT)

                    # query_sm = E * (1/colsum), written straight to SBUF
                    rcp = rcpp.tile([C, GRP], FP32, tag="rcp")
                    nc.vector.reciprocal_approx_fast(
                        out=rcp[:, 0:gw], in_=cs[:, 0:gw])
                    nc.vector.tensor_mul(
                        qsm_nat[:, bass.ds(off + g0, gw)],
                        en[:, bass.ds(g0, gw)],
                        rcp[:, 0:gw],
                    )

                if pending is not None:
                    emit_ctx(*pending)
                pending = (eTs, vT, Ji)
                off += SW
            emit_ctx(*pending)

            # ---- block-diagonal context weights ----
            rowsum = smalls.tile([C, 1], FP32, tag="rowsum")
            nc.vector.tensor_reduce(
                rowsum[:], rs_acc[:], mybir.AxisListType.X,
                mybir.AluOpType.add)
            rs_rcp = smalls.tile([C, 1], FP32, tag="rs_rcp")
            nc.vector.reciprocal(rs_rcp[:], rowsum[:])
            scaled = smalls.tile([C, C], BF16, tag="scaled")
            nc.vector.tensor_scalar(
                scaled[:], ctx_ps[:], rs_rcp[:, 0:1], None,
                mybir.AluOpType.mult,
            )
            bd = smalls.tile([C, C], BF16, tag="bd")
            nc.vector.tensor_mul(bd[:], scaled[:], bd8[:])

            # ---- attended tiles: matmul, cast, store ----
            for t in range(NOT):
                att = ps_mix.tile([C, OT], FP32, tag="mix")
                for q in range(OT // QS):
                    nc.tensor.matmul(
                        att[:, bass.ts(q, QS)], bd[:],
                        qsm_nat[:, bass.ds(t * OT + q * QS, QS)],
                    )
                ot = outp.tile([C, OT], BF16, tag="ot")
                if t % 2 == 0:
                    nc.scalar.copy(ot[:], att[:])
                else:
                    nc.vector.tensor_copy(ot[:], att[:])
                nc.scalar.dma_start(out=out[:, bass.ts(t, OT)], in_=ot[:])

    nc.compile()
    return nc


def _get_nc():
    if "nc" not in _cache:
        _cache["nc"] = _build()
    return _cache["nc"]


def _bd8_np() -> np.ndarray:
    import ml_dtypes

    m = np.zeros((C, C), dtype=np.float32)
    for h in range(HEADS):
        m[h * HC:(h + 1) * HC, h * HC:(h + 1) * HC] = 1.0
    return m.astype(ml_dtypes.bfloat16)


def _ident_np() -> np.ndarray:
    import ml_dtypes

    return np.eye(C, dtype=np.float32).astype(ml_dtypes.bfloat16)


def _to_np(a) -> np.ndarray:
    """Materialize to float32 numpy; retry once on a transient bad fetch
    (device-backed arrays have been observed to materialize NaNs once)."""
    out = np.asarray(a, dtype=np.float32)
    if np.isnan(out).any():
        out = np.asarray(a, dtype=np.float32)
    return out


def _in_maps(x1: np.ndarray, x2: np.ndarray) -> list[dict]:
    """Host-side sharding + layout: x2 natural, x1 A-layout, bf16."""
    import ml_dtypes

    BF = ml_dtypes.bfloat16
    x1 = _to_np(x1).reshape(B, C, N)
    x2 = _to_np(x2).reshape(B, C, N)
    x2b = x2.astype(BF)
    # A-layout: arr[b, p, j, c] = x1[b, c, j*128 + p]
    x1a = np.ascontiguousarray(
        x1.reshape(B, C, J, 128).transpose(0, 3, 2, 1)).astype(BF)
    x1a = x1a.reshape(B, 128, N)
    bd8 = _bd8_np()
    ident = _ident_np()
    return [
        {"x2n": x2b[i], "x1p": x1a[i], "bd8": bd8, "ident": ident}
        for i in range(NCORES)
    ]


def kernel(x1: np.ndarray, x2: np.ndarray) -> np.ndarray:
    from concourse.bass_utils import run_bass_kernel_spmd

    nc = _get_nc()
    in_maps = _in_maps(x1, x2)
    res = run_bass_kernel_spmd(nc, in_maps, core_ids=list(range(NCORES)))
    outs = [np.asarray(res.results[i]["out"], dtype=np.float32) for i in range(NCORES)]
    return np.stack(outs, axis=0).reshape(B, C, H, W)


# revision 22
# speedup vs baseline: 1.1057x; 1.1057x over previous
"""Trainium2 Bass kernel for cross "efficient attention".

Reference computation (per batch b, head h, with C=128, HEADS=8, hc=16, n=16384):
    k = x2[b].reshape(HEADS, hc, n); v = x1[b].reshape(HEADS, hc, n)
    key_sm   = softmax(k, axis=-1)          # over n
    query_sm = softmax(k, axis=1)           # over hc (head channels)
    context  = key_sm @ v^T                 # (hc, hc)
    out[b,h] = context^T @ query_sm         # (hc, n)

Sharding: data-parallel over batch B=8 across the 8 NeuronCores (no
collectives).  Inputs are ~N(0,1) so softmax needs no max-subtraction.

Layouts: x2 arrives natural [C, N] bf16; x1 arrives in "A-layout"
[128, N] bf16 (partition p holds n = j*128+p for every channel-block
j) so it is directly the n-on-partitions contraction operand.  All
DMAs are contiguous per partition.  HBM traffic 12 MiB/core.

Single streaming pipeline per slab of n:
    E    = exp(x2)               + rowsum accum        (scalar)
    eT   = PE-transpose(E)       -> SBUF               (PE + s/v copies)
    ctx += eT_j^T @ vT_j         (lagged 1 slab)       (PE)
    cs   = bd8 @ E               per-head colsums,
                                 broadcast for free    (PE -> PSUM)
    qsm  = E / cs                                      (vector divide)
Then bd = blockdiag(ctx * 1/rowsum) and attended tiles
    out[:, t] = bd^T @ qsm[:, t]                       (PE + s/v casts)
stream out.  The colsum and attended tiles share one PSUM pool ring so
the attended matmuls chain naturally behind the last colsum with no
pool-close barrier.
"""

import numpy as np
from contextlib import ExitStack

B, C, H, W = 8, 128, 128, 128
N = H * W                 # 16384
J = N // 128              # 128 channel-blocks
HEADS, HC = 8, 16
NCORES = 8

SLABS = [2048] * 7 + [1024, 512, 512]
assert sum(SLABS) == N
NSLAB = len(SLABS)
GRP = 1024                # transpose / colsum / divide chunk (PSUM bank-pair)
QS = 512                  # matmul moving-operand tile (one PSUM bank f32)
OT = 1024                 # attended tile width
NOT = N // OT             # 16

_cache: dict = {}


def _build():
    import concourse.bass as bass
    import concourse.tile as tile
    from concourse import bacc, mybir

    FP32 = mybir.dt.float32
    BF16 = mybir.dt.bfloat16
    AF = mybir.ActivationFunctionType

    nc = bacc.Bacc("TRN2", target_bir_lowering=False, debug=False)

    x2n = nc.dram_tensor("x2n", [C, N], BF16, kind="ExternalInput")
    x1p = nc.dram_tensor("x1p", [128, N], BF16, kind="ExternalInput")
    bd8_in = nc.dram_tensor("bd8", [C, C], BF16, kind="ExternalInput")
    ident_in = nc.dram_tensor("ident", [C, C], BF16, kind="ExternalInput")
    out = nc.dram_tensor("out", [C, N], BF16, kind="ExternalOutput")

    with tile.TileContext(nc) as tc:
        with ExitStack() as ctx:
            persist = ctx.enter_context(tc.tile_pool(name="persist", bufs=1))
            x2ld = ctx.enter_context(tc.tile_pool(name="x2ld", bufs=4))
            vld = ctx.enter_context(tc.tile_pool(name="vld", bufs=4))
            enp = ctx.enter_context(tc.tile_pool(name="enp", bufs=4))
            eTp = ctx.enter_context(tc.tile_pool(name="eTp", bufs=6))
            rcpp = ctx.enter_context(tc.tile_pool(name="rcpp", bufs=3))
            outp = ctx.enter_context(tc.tile_pool(name="outp", bufs=4))
            smalls = ctx.enter_context(tc.tile_pool(name="smalls", bufs=1))

            qsm_nat = persist.tile([C, N], BF16, tag="qsm_nat")
            bd8 = smalls.tile([C, C], BF16, tag="bd8")
            ident = smalls.tile([C, C], BF16, tag="ident")
            rs_acc = smalls.tile([C, NSLAB], FP32, tag="rs_acc")

            ps_tr = ctx.enter_context(
                tc.tile_pool(name="pstr", bufs=3, space="PSUM"))
            ps_ctx = ctx.enter_context(
                tc.tile_pool(name="psctx", bufs=1, space="PSUM"))
            # colsum tiles and attended tiles share this ring: the first
            # attended matmul naturally chains behind the last colsum's
            # consumer instead of a pool-close barrier.
            ps_mix = ctx.enter_context(
                tc.tile_pool(name="psmix", bufs=2, space="PSUM"))

            ctx_ps = ps_ctx.tile([C, 128], FP32, tag="ctx")

            mm_idx = 0
            pending = None       # (eT tiles, vT, n_chunks) of previous slab
            cpy = 0              # round-robin for PSUM->SBUF copy engine

            def emit_ctx(eTs, vT, nch):
                nonlocal mm_idx
                for j in range(nch):
                    nc.tensor.matmul(
                        ctx_ps[:],
                        eTs[j // 8][:, bass.ts(j % 8, 128)],
                        vT[:, bass.ts(j, 128)],
                        start=(mm_idx == 0),
                        stop=(mm_idx == J - 1),
                    )
                    mm_idx += 1

            off = 0
            for i, SW in enumerate(SLABS):
                Ji = SW // 128
                x2t = x2ld.tile([C, SW], BF16, tag="x2t")
                nc.sync.dma_start(out=x2t[:], in_=x2n[:, bass.ds(off, SW)])
                vT = vld.tile([128, SW], BF16, tag="vT")
                nc.sync.dma_start(out=vT[:], in_=x1p[:, bass.ds(off, SW)])
                if i == 0:
                    nc.scalar.dma_start(out=ident[:], in_=ident_in[:])
                    nc.scalar.dma_start(out=bd8[:], in_=bd8_in[:])

                en = enp.tile([C, SW], BF16, tag="en")
                nc.scalar.activation(
                    en[:], x2t[:], AF.Exp, accum_out=rs_acc[:, i:i + 1])

                # per-head colsums on the PE: bd8 is block-diagonal ones, so
                # the result arrives already broadcast to all 16 channels.
                eTs = []
                for g0 in range(0, SW, GRP):
                    gw = min(GRP, SW - g0)
                    cs = ps_mix.tile([C, GRP], FP32, tag="mix")
                    for q0 in range(0, gw, QS):
                        qw = min(QS, gw - q0)
                        nc.tensor.matmul(
                            cs[:, bass.ds(q0, qw)], bd8[:],
                            en[:, bass.ds(g0 + q0, qw)],
                        )
                    # transposes of the same chunk -> eT for the ctx matmul
                    te = ps_tr.tile([C, GRP], BF16, tag="te")
                    for j in range(gw // 128):
                        nc.tensor.transpose(
                            te[:, bass.ts(j, 128)],
                            en[:, bass.ds(g0 + j * 128, 128)],
                            ident[:],
                        )
                    eT = eTp.tile([C, GRP], BF16, tag="eT")
                    if cpy % 2 == 0:
                        nc.scalar.copy(eT[:, 0:gw], te[:, 0:gw])
                    else:
                        nc.vector.tensor_copy(eT[:, 0:gw], te[:, 0:gw])
                    cpy += 1
                    eTs.append(eT)

                    # query_sm = E * (1/colsum), written straight to SBUF.
                    # The reciprocal is DVE-only; the multiplies alternate
                    # between vector and the otherwise-idle gpsimd.
                    rcp = rcpp.tile([C, GRP], FP32, tag="rcp")
                    nc.vector.reciprocal_approx_fast(
                        out=rcp[:, 0:gw], in_=cs[:, 0:gw])
                    meng = nc.gpsimd if (g0 // GRP) % 2 == 1 else nc.vector
                    meng.tensor_mul(
                        qsm_nat[:, bass.ds(off + g0, gw)],
                        en[:, bass.ds(g0, gw)],
                        rcp[:, 0:gw],
                    )

                if pending is not None:
                    emit_ctx(*pending)
                pending = (eTs, vT, Ji)
                off += SW
            emit_ctx(*pending)

            # ---- block-diagonal context weights ----
            rowsum = smalls.tile([C, 1], FP32, tag="rowsum")
            nc.vector.tensor_reduce(
                rowsum[:], rs_acc[:], mybir.AxisListType.X,
                mybir.AluOpType.add)
            rs_rcp = smalls.tile([C, 1], FP32, tag="rs_rcp")
            nc.vector.reciprocal(rs_rcp[:], rowsum[:])
            scaled = smalls.tile([C, C], BF16, tag="scaled")
            nc.vector.tensor_scalar(
                scaled[:], ctx_ps[:], rs_rcp[:, 0:1], None,
                mybir.AluOpType.mult,
            )
            bd = smalls.tile([C, C], BF16, tag="bd")
            nc.vector.tensor_mul(bd[:], scaled[:], bd8[:])

            # ---- attended tiles: matmul, cast, store ----
            for t in range(NOT):
                att = ps_mix.tile([C, OT], FP32, tag="mix")
                for q in range(OT // QS):
                    nc.tensor.matmul(
                        att[:, bass.ts(q, QS)], bd[:],
                        qsm_nat[:, bass.ds(t * OT + q * QS, QS)],
                    )
                ot = outp.tile([C, OT], BF16, tag="ot")
                if t % 2 == 0:
                    nc.scalar.copy(ot[:], att[:])
                else:
                    nc.vector.tensor_copy(ot[:], att[:])
                nc.sync.dma_start(out=out[:, bass.ts(t, OT)], in_=ot[:])

    nc.compile()
    return nc


def _get_nc():
    if "nc" not in _cache:
        _cache["nc"] = _build()
    return _cache["nc"]


def _bd8_np() -> np.ndarray:
    import ml_dtypes

    m = np.zeros((C, C), dtype=np.float32)
    for h in range(HEADS):
        m[h * HC:(h + 1) * HC, h * HC:(h + 1) * HC] = 1.0
    return m.astype(ml_dtypes.bfloat16)


def _ident_np() -> np.ndarray:
    import ml_dtypes

    return np.eye(C, dtype=np.float32).astype(ml_dtypes.bfloat16)


def _to_np(a) -> np.ndarray:
    """Materialize to float32 numpy; retry once on a transient bad fetch
    (device-backed arrays have been observed to materialize NaNs once)."""
    out = np.asarray(a, dtype=np.float32)
    if np.isnan(out).any():
        out = np.asarray(a, dtype=np.float32)
    return out


def _in_maps(x1: np.ndarray, x2: np.ndarray) -> list[dict]:
    """Host-side sharding + layout: x2 natural, x1 A-layout, bf16."""
    import ml_dtypes

    BF = ml_dtypes.bfloat16
    x1 = _to_np(x1).reshape(B, C, N)
    x2 = _to_np(x2).reshape(B, C, N)
    x2b = x2.astype(BF)
    # A-layout: arr[b, p, j, c] = x1[b, c, j*128 + p]
    x1a = np.ascontiguousarray(
        x1.reshape(B, C, J, 128).transpose(0, 3, 2, 1)).astype(BF)
    x1a = x1a.reshape(B, 128, N)
    bd8 = _bd8_np()
    ident = _ident_np()
    return [
        {"x2n": x2b[i], "x1p": x1a[i], "bd8": bd8, "ident": ident}
        for i in range(NCORES)
    ]


def kernel(x1: np.ndarray, x2: np.ndarray) -> np.ndarray:
    from concourse.bass_utils import run_bass_kernel_spmd

    nc = _get_nc()
    in_maps = _in_maps(x1, x2)
    res = run_bass_kernel_spmd(nc, in_maps, core_ids=list(range(NCORES)))
    outs = [np.asarray(res.results[i]["out"], dtype=np.float32) for i in range(NCORES)]
    return np.stack(outs, axis=0).reshape(B, C, H, W)


# revision 26
# speedup vs baseline: 1.1457x; 1.0362x over previous
"""Trainium2 Bass kernel for cross "efficient attention".

Reference computation (per batch b, head h, with C=128, HEADS=8, hc=16, n=16384):
    k = x2[b].reshape(HEADS, hc, n); v = x1[b].reshape(HEADS, hc, n)
    key_sm   = softmax(k, axis=-1)          # over n
    query_sm = softmax(k, axis=1)           # over hc (head channels)
    context  = key_sm @ v^T                 # (hc, hc)
    out[b,h] = context^T @ query_sm         # (hc, n)

Sharding: data-parallel over batch B=8 across the 8 NeuronCores (no
collectives).  Inputs are ~N(0,1) so softmax needs no max-subtraction.

Layouts: x2 arrives natural [C, N] bf16; x1 arrives in "A-layout"
[128, N] bf16 (partition p holds n = j*128+p for every channel-block
j) so it is directly the n-on-partitions contraction operand.  All
DMAs are contiguous per partition.  HBM traffic 12 MiB/core.

Single streaming pipeline per slab of n:
    E    = exp(x2)               + rowsum accum        (scalar)
    eT   = PE-transpose(E)       -> SBUF               (PE + s/v copies)
    ctx += eT_j^T @ vT_j         (lagged 1 slab)       (PE)
    cs   = bd8 @ E               per-head colsums,
                                 broadcast for free    (PE -> PSUM)
    qsm  = E / cs                                      (vector divide)
Then bd = blockdiag(ctx * 1/rowsum) and attended tiles
    out[:, t] = bd^T @ qsm[:, t]                       (PE + s/v casts)
stream out.  The colsum and attended tiles share one PSUM pool ring so
the attended matmuls chain naturally behind the last colsum with no
pool-close barrier.
"""

import numpy as np
from contextlib import ExitStack

B, C, H, W = 8, 128, 128, 128
N = H * W                 # 16384
J = N // 128              # 128 channel-blocks
HEADS, HC = 8, 16
NCORES = 8

SLABS = [2048] * 7 + [1024, 512, 512]
assert sum(SLABS) == N
NSLAB = len(SLABS)
GRP = 1024                # transpose / colsum / divide chunk (PSUM bank-pair)
QS = 512                  # matmul moving-operand tile (one PSUM bank f32)
OT = 1024                 # attended tile width
NOT = N // OT             # 16

_cache: dict = {}


def _build():
    import concourse.bass as bass
    import concourse.tile as tile
    from concourse import bacc, mybir

    FP32 = mybir.dt.float32
    BF16 = mybir.dt.bfloat16
    AF = mybir.ActivationFunctionType

    nc = bacc.Bacc("TRN2", target_bir_lowering=False, debug=False)

    x2n = nc.dram_tensor("x2n", [C, N], BF16, kind="ExternalInput")
    x1p = nc.dram_tensor("x1p", [128, N], BF16, kind="ExternalInput")
    bd8_in = nc.dram_tensor("bd8", [C, C], BF16, kind="ExternalInput")
    ident_in = nc.dram_tensor("ident", [C, C], BF16, kind="ExternalInput")
    out = nc.dram_tensor("out", [C, N], BF16, kind="ExternalOutput")

    with tile.TileContext(nc) as tc:
        with ExitStack() as ctx:
            persist = ctx.enter_context(tc.tile_pool(name="persist", bufs=1))
            x2ld = ctx.enter_context(tc.tile_pool(name="x2ld", bufs=4))
            vld = ctx.enter_context(tc.tile_pool(name="vld", bufs=4))
            enp = ctx.enter_context(tc.tile_pool(name="enp", bufs=6))
            eTp = ctx.enter_context(tc.tile_pool(name="eTp", bufs=6))
            rcpp = ctx.enter_context(tc.tile_pool(name="rcpp", bufs=6))
            outp = ctx.enter_context(tc.tile_pool(name="outp", bufs=4))
            smalls = ctx.enter_context(tc.tile_pool(name="smalls", bufs=1))

            qsm_nat = persist.tile([C, N], BF16, tag="qsm_nat")
            bd8 = smalls.tile([C, C], BF16, tag="bd8")
            ident = smalls.tile([C, C], BF16, tag="ident")
            rs_acc = smalls.tile([C, NSLAB], FP32, tag="rs_acc")

            ps_tr = ctx.enter_context(
                tc.tile_pool(name="pstr", bufs=3, space="PSUM"))
            ps_ctx = ctx.enter_context(
                tc.tile_pool(name="psctx", bufs=1, space="PSUM"))
            # colsum tiles and attended tiles share this ring: the first
            # attended matmul naturally chains behind the last colsum's
            # consumer instead of a pool-close barrier.
            ps_mix = ctx.enter_context(
                tc.tile_pool(name="psmix", bufs=2, space="PSUM"))

            ctx_ps = ps_ctx.tile([C, 128], FP32, tag="ctx")

            mm_idx = 0
            pending = None       # (eT tiles, vT, n_chunks) of previous slab
            cpy = 0              # round-robin for PSUM->SBUF copy engine

            def emit_ctx(eTs, vT, nch):
                nonlocal mm_idx
                for j in range(nch):
                    nc.tensor.matmul(
                        ctx_ps[:],
                        eTs[j // 8][:, bass.ts(j % 8, 128)],
                        vT[:, bass.ts(j, 128)],
                        start=(mm_idx == 0),
                        stop=(mm_idx == J - 1),
                    )
                    mm_idx += 1

            off = 0
            for i, SW in enumerate(SLABS):
                Ji = SW // 128
                x2t = x2ld.tile([C, SW], BF16, tag="x2t")
                nc.sync.dma_start(out=x2t[:], in_=x2n[:, bass.ds(off, SW)])
                vT = vld.tile([128, SW], BF16, tag="vT")
                nc.sync.dma_start(out=vT[:], in_=x1p[:, bass.ds(off, SW)])
                if i == 0:
                    nc.scalar.dma_start(out=ident[:], in_=ident_in[:])
                    nc.scalar.dma_start(out=bd8[:], in_=bd8_in[:])

                en = enp.tile([C, SW], BF16, tag="en")
                nc.scalar.activation(
                    en[:], x2t[:], AF.Exp, accum_out=rs_acc[:, i:i + 1])

                # per-head colsums on the PE: bd8 is block-diagonal ones, so
                # the result arrives already broadcast to all 16 channels.
                eTs = []
                for g0 in range(0, SW, GRP):
                    gw = min(GRP, SW - g0)
                    cs = ps_mix.tile([C, GRP], FP32, tag="mix")
                    for q0 in range(0, gw, QS):
                        qw = min(QS, gw - q0)
                        nc.tensor.matmul(
                            cs[:, bass.ds(q0, qw)], bd8[:],
                            en[:, bass.ds(g0 + q0, qw)],
                        )
                    # transposes of the same chunk -> eT for the ctx matmul
                    te = ps_tr.tile([C, GRP], BF16, tag="te")
                    for j in range(gw // 128):
                        nc.tensor.transpose(
                            te[:, bass.ts(j, 128)],
                            en[:, bass.ds(g0 + j * 128, 128)],
                            ident[:],
                        )
                    # eT copies stay on scalar: the ctx->bd chain must never
                    # queue behind vector's recip/mult backlog.
                    eT = eTp.tile([C, GRP], BF16, tag="eT")
                    nc.scalar.copy(eT[:, 0:gw], te[:, 0:gw])
                    eTs.append(eT)

                    # query_sm = E * (1/colsum), written straight to SBUF.
                    # The reciprocal is DVE-only; the multiplies alternate
                    # between vector and the otherwise-idle gpsimd.
                    rcp = rcpp.tile([C, GRP], FP32, tag="rcp")
                    nc.vector.reciprocal_approx_fast(
                        out=rcp[:, 0:gw], in_=cs[:, 0:gw])
                    meng = nc.gpsimd if i < 6 else nc.vector
                    meng.tensor_mul(
                        qsm_nat[:, bass.ds(off + g0, gw)],
                        en[:, bass.ds(g0, gw)],
                        rcp[:, 0:gw],
                    )

                if pending is not None:
                    emit_ctx(*pending)
                pending = (eTs, vT, Ji)
                off += SW
            emit_ctx(*pending)

            # ---- block-diagonal context weights ----
            rowsum = smalls.tile([C, 1], FP32, tag="rowsum")
            nc.vector.tensor_reduce(
                rowsum[:], rs_acc[:], mybir.AxisListType.X,
                mybir.AluOpType.add)
            rs_rcp = smalls.tile([C, 1], FP32, tag="rs_rcp")
            nc.vector.reciprocal(rs_rcp[:], rowsum[:])
            scaled = smalls.tile([C, C], BF16, tag="scaled")
            nc.vector.tensor_scalar(
                scaled[:], ctx_ps[:], rs_rcp[:, 0:1], None,
                mybir.AluOpType.mult,
            )
            bd = smalls.tile([C, C], BF16, tag="bd")
            nc.vector.tensor_mul(bd[:], scaled[:], bd8[:])

            # ---- attended tiles: matmul, cast, store ----
            for t in range(NOT):
                att = ps_mix.tile([C, OT], FP32, tag="mix")
                for q in range(OT // QS):
                    nc.tensor.matmul(
                        att[:, bass.ts(q, QS)], bd[:],
                        qsm_nat[:, bass.ds(t * OT + q * QS, QS)],
                    )
                # casts mostly on scalar; vector (whose backlog drains last)
                # only takes the final tiles, which run latest anyway
                ot = outp.tile([C, OT], BF16, tag="ot")
                if t < 11:
                    nc.scalar.copy(ot[:], att[:])
                else:
                    nc.vector.tensor_copy(ot[:], att[:])
                nc.sync.dma_start(out=out[:, bass.ts(t, OT)], in_=ot[:])

    nc.compile()
    return nc


def _get_nc():
    if "nc" not in _cache:
        _cache["nc"] = _build()
    return _cache["nc"]


def _bd8_np() -> np.ndarray:
    import ml_dtypes

    m = np.zeros((C, C), dtype=np.float32)
    for h in range(HEADS):
        m[h * HC:(h + 1) * HC, h * HC:(h + 1) * HC] = 1.0
    return m.astype(ml_dtypes.bfloat16)


def _ident_np() -> np.ndarray:
    import ml_dtypes

    return np.eye(C, dtype=np.float32).astype(ml_dtypes.bfloat16)


def _to_np(a) -> np.ndarray:
    """Materialize to float32 numpy; retry once on a transient bad fetch
    (device-backed arrays have been observed to materialize NaNs once)."""
    out = np.asarray(a, dtype=np.float32)
    if np.isnan(out).any():
        out = np.asarray(a, dtype=np.float32)
    return out


def _in_maps(x1: np.ndarray, x2: np.ndarray) -> list[dict]:
    """Host-side sharding + layout: x2 natural, x1 A-layout, bf16."""
    import ml_dtypes

    BF = ml_dtypes.bfloat16
    x1 = _to_np(x1).reshape(B, C, N)
    x2 = _to_np(x2).reshape(B, C, N)
    x2b = x2.astype(BF)
    # A-layout: arr[b, p, j, c] = x1[b, c, j*128 + p]
    x1a = np.ascontiguousarray(
        x1.reshape(B, C, J, 128).transpose(0, 3, 2, 1)).astype(BF)
    x1a = x1a.reshape(B, 128, N)
    bd8 = _bd8_np()
    ident = _ident_np()
    return [
        {"x2n": x2b[i], "x1p": x1a[i], "bd8": bd8, "ident": ident}
        for i in range(NCORES)
    ]


def kernel(x1: np.ndarray, x2: np.ndarray) -> np.ndarray:
    from concourse.bass_utils import run_bass_kernel_spmd

    nc = _get_nc()
    in_maps = _in_maps(x1, x2)
    res = run_bass_kernel_spmd(nc, in_maps, core_ids=list(range(NCORES)))
    outs = [np.asarray(res.results[i]["out"], dtype=np.float32) for i in range(NCORES)]
    return np.stack(outs, axis=0).reshape(B, C, H, W)
